# revision 1
# baseline (speedup 1.0000x reference)
"""Multi-head attention (B=2, S=2048, D=1024, H=16, causal mask) on 8 TRN2
NeuronCores, head-parallel: each core computes 2 heads' q/k/v + attention and
a partial output projection; host sums the 8 partials and adds bo.

Layouts (per core):
  xT      (1024, 4096)  feature-major tokens (b-major), replicated
  wqkvT   (1024, 384)   [wq(/8) | wk | wv] columns for this core's 2 heads
  bqkv    (128, 3)      per-dim biases (bq/8, bk, bv)
  woT     (128, 1024)   wo rows for this core's head dims
  out     (4096, 1024)  partial contribution (host sums over cores, adds bo)

All matmuls run as float32r (TF32-like, full PE rate at moving dim >= 256).
Scores are computed transposed (S^T[k, q]) so softmax needs no transposes:
P^T = exp(S^T) unnormalized (max-subtraction skipped; scores bounded ~10 for
this problem's scale), causal zeroing via gpsimd affine_select, denominators
from a ones-column appended to v. 1/denom rows are broadcast across
partitions with a tiny ones-matmul and attnT is normalized in place, which
makes the output projection a single K=128 matmul per chunk (heads sum).
"""

import numpy as np

import concourse.bass as bass
import concourse.tile as tile
from concourse import bacc, mybir
from concourse.bass_utils import run_bass_kernel_spmd

B, S, D, H = 2, 2048, 1024, 16
DH = D // H  # 64
NCORES = 8
HPC = H // NCORES  # 2 heads per core
T = B * S  # 4096
QCH = 512  # q-chunk (moving dim)
KCH = 128  # k-chunk (stationary dim)
NQC = S // QCH  # 4 per batch
NKC = S // KCH  # 16 per batch
NTC = T // QCH  # 8 token chunks overall
ND = D // 128  # 8 feature chunks

f32 = mybir.dt.float32
f32r = mybir.dt.float32r
AF = mybir.ActivationFunctionType
ALU = mybir.AluOpType

# score->exp->PV software pipeline depth (in k-chunks)
PIPE = 3


def _classify_blocks(mask):
    """mask: (S, S) bool [q, k]. Returns dict (qc, kc) -> ('none'|'all'|'causal'|'mixed', packed_idx)."""
    blocks = {}
    qg, kg = np.meshgrid(np.arange(S), np.arange(S), indexing="ij")
    causal = qg >= kg
    n_mixed = 0
    for qc in range(NQC):
        for kc in range(NKC):
            reg = mask[qc * QCH : (qc + 1) * QCH, kc * KCH : (kc + 1) * KCH]
            if not reg.any():
                blocks[(qc, kc)] = ("none", -1)
            elif reg.all():
                blocks[(qc, kc)] = ("all", -1)
            elif np.array_equal(
                reg, causal[qc * QCH : (qc + 1) * QCH, kc * KCH : (kc + 1) * KCH]
            ):
                blocks[(qc, kc)] = ("causal", -1)
            else:
                blocks[(qc, kc)] = ("mixed", n_mixed)
                n_mixed += 1
    return blocks, n_mixed


def _build(mask, reps=1):
    blocks, n_mixed = _classify_blocks(mask)

    nc = bacc.Bacc("TRN2", target_bir_lowering=False, debug=False, num_devices=NCORES)
    xt_d = nc.dram_tensor("xt", (D, T), f32r, kind="ExternalInput").ap()
    w_d = nc.dram_tensor("wqkv", (D, 3 * 128), f32r, kind="ExternalInput").ap()
    b_d = nc.dram_tensor("bqkv", (128, 3), f32, kind="ExternalInput").ap()
    wo_d = nc.dram_tensor("wot", (128, D), f32r, kind="ExternalInput").ap()
    id_d = nc.dram_tensor("ident", (128, 64), f32r, kind="ExternalInput").ap()
    out_d = nc.dram_tensor("out", (T, D), f32, kind="ExternalOutput").ap()
    if n_mixed:
        mb_d = nc.dram_tensor(
            "mblk", (n_mixed * 128, QCH), f32r, kind="ExternalInput"
        ).ap()

    with tile.TileContext(nc) as tc:
        with (
            tc.tile_pool(name="const", bufs=1) as cpool,
            tc.tile_pool(name="act", bufs=1) as apool,
            tc.tile_pool(name="work", bufs=1) as wpool,
            tc.tile_pool(name="psum", bufs=1, space="PSUM") as ppool,
        ):
            # ---- constants ----
            # interleave weight-chunk and first-x-chunk DMAs so the first
            # projection matmuls start as soon as (w0, x0_0) land
            x0cell = []
            wtiles = []
            for dc in range(ND):
                wt = cpool.tile([128, 384], f32r, name=f"w{dc}")
                nc.sync.dma_start(wt[:], w_d[dc * 128 : (dc + 1) * 128, :])
                wtiles.append(wt)
                x0 = wpool.tile([128, QCH], f32r, tag="x", bufs=16, name=f"x0_{dc}")
                nc.sync.dma_start(x0[:], xt_d[dc * 128 : (dc + 1) * 128, 0:QCH])
                x0cell.append(x0)
            bqkv = cpool.tile([128, 3], f32)
            nc.sync.dma_start(bqkv[:], b_d)
            ident = cpool.tile([128, 64], f32r)
            nc.sync.dma_start(ident[:], id_d)
            wot = cpool.tile([128, D], f32r)
            nc.sync.dma_start(wot[:], wo_d)
            ones16 = cpool.tile([128, NKC], f32)
            nc.vector.memset(ones16[:], 1.0)
            ones64f = cpool.tile([1, 64], f32)
            nc.vector.memset(ones64f[:], 1.0)
            ones64 = cpool.tile([1, 64], f32r)
            nc.vector.tensor_copy(ones64[:], ones64f[:])
            zero384f = cpool.tile([128, 384], f32)
            nc.vector.memset(zero384f[:], 0.0)
            zero384 = cpool.tile([128, 384], f32r)
            nc.vector.tensor_copy(zero384[:], zero384f[:])

            # ---- per-batch persistent activations ----
            qT = [
                apool.tile([128, S], f32r, tag=f"qT{b}", name=f"qT{b}")
                for b in range(B)
            ]
            kT = [
                apool.tile([128, S], f32r, tag=f"kT{b}", name=f"kT{b}")
                for b in range(B)
            ]
            vaug = [
                [
                    apool.tile([128, NKC * 65], f32r, tag=f"va{b}{h}", name=f"va{b}{h}")
                    for h in range(HPC)
                ]
                for b in range(B)
            ]
            for b in range(B):
                for h in range(HPC):
                    nc.vector.tensor_copy(vaug[b][h][:, 64 :: 65], ones16[:])

            for _rep in range(reps):
                # ---- phase A: qkv projections + v transpose (pipelined) ----
                prev_vst = [None]

                def emit_vtrans(vst, b, tq):
                    for j in range(QCH // 128):
                        kc = tq * 4 + j
                        for h in range(HPC):
                            tp = ppool.tile(
                                [128, 512], f32r, tag="acc", bufs=2, name=f"vt{b}{tq}{j}{h}"
                            )
                            nc.tensor.transpose(
                                tp[:, 0:64],
                                vst[h * 64 : (h + 1) * 64, j * 128 : (j + 1) * 128],
                                ident[h * 64 : (h + 1) * 64, :],
                                tile_position=(h * 64, 0),
                            )
                            nc.vector.tensor_copy(
                                vaug[b][h][:, kc * 65 : kc * 65 + 64], tp[:, 0:64]
                            )

                def unit_x(t, cell):
                    for dc in range(ND):
                        xt = wpool.tile(
                            [128, QCH], f32r, tag="x", bufs=16, name=f"x{t}_{dc}"
                        )
                        nc.sync.dma_start(
                            xt[:], xt_d[dc * 128 : (dc + 1) * 128, t * QCH : (t + 1) * QCH]
                        )
                        cell.append(xt)

                def unit_proj(t, p, cell):
                    b, tq = t // NQC, t % NQC
                    ps = ppool.tile(
                        [128, QCH], f32, tag="st", bufs=5, name=f"pj{t}_{p}"
                    )
                    for dc in range(ND):
                        nc.tensor.matmul(
                            ps[:],
                            wtiles[dc][:, p * 128 : (p + 1) * 128],
                            cell[dc][:],
                            start=(dc == 0),
                            stop=(dc == ND - 1),
                        )
                    if p == 0:
                        dst = qT[b][:, tq * QCH : (tq + 1) * QCH]
                    elif p == 1:
                        dst = kT[b][:, tq * QCH : (tq + 1) * QCH]
                    else:
                        dst = wpool.tile(
                            [128, QCH], f32r, tag="vst", bufs=2, name=f"vst{t}"
                        )[:]
                    nc.vector.tensor_scalar_add(dst, ps[:], bqkv[:, p : p + 1])
                    if p == 2:
                        prev_vst[0] = (dst, b, tq)

                def unit_vtrans(t):
                    if prev_vst[0] is not None:
                        emit_vtrans(*prev_vst[0])
                        prev_vst[0] = None

                # fine-grained projection units, drained into the (exp-bound)
                # attention k-loops; need(t, n) enforces the semantic order
                # (writers of qT/kT/vaug before their readers)
                fill = []  # list of (chunk, unit_idx, closure)

                def queue_A(t, _first=(_rep == 0)):
                    if t >= NTC:
                        return
                    if t == 0 and _first:
                        # chunk 0's x DMAs were pre-emitted with the weights
                        cell = x0cell
                        fill.append((t, 0, lambda: None))
                    else:
                        cell = []
                        fill.append((t, 0, lambda t=t, cell=cell: unit_x(t, cell)))
                    for p in range(3):
                        fill.append(
                            (t, 1 + p, lambda t=t, p=p, cell=cell: unit_proj(t, p, cell))
                        )
                    fill.append((t, 4, lambda t=t: unit_vtrans(t)))

                def need_A(t, n_units):
                    # drain all units of chunks < t, plus first n_units of t
                    while fill and (fill[0][0] < t or (fill[0][0] == t and fill[0][1] < n_units)):
                        fill.pop(0)[2]()

                def pop_fill():
                    if fill:
                        fill.pop(0)[2]()

                queued = [0]

                def ensure_queued(t):
                    while queued[0] <= min(t, NTC - 1):
                        queue_A(queued[0])
                        queued[0] += 1

                # ---- phases B/C per (batch, q-chunk) ----
                pending = []  # deferred phase-C emitters from the previous q-chunk

                def flush_pending():
                    while pending:
                        pending.pop(0)()

                for b in range(B):
                    for qc in range(NQC):
                        kcs = [kc for kc in range(NKC) if blocks[(qc, kc)][0] != "none"]
                        # scores/PV read kT/vaug chunks up to max(kcs) — for a
                        # causal mask this is just qc, but a generic mask can
                        # attend ahead of the diagonal
                        kmax = max(kcs) // (QCH // KCH) if kcs else 0
                        req = b * NQC + max(qc, kmax)
                        ensure_queued(req)
                        need_A(req, 3)  # x + q/k projections must precede scores
                        acc = [
                            ppool.tile(
                                [128, QCH], f32, tag="acc", bufs=2, name=f"acc{b}_{qc}_{h}"
                            )
                            for h in range(HPC)
                        ]
                        pts = {}

                        def emit_scores(i, b=b, qc=qc, kcs=kcs, pts=pts):
                            kc = kcs[i]
                            kind, midx = blocks[(qc, kc)]
                            for h in range(HPC):
                                st = ppool.tile(
                                    [128, QCH],
                                    f32,
                                    tag="st",
                                    bufs=5,
                                    name=f"st{b}_{qc}_{i}_{h}",
                                )
                                f0 = 0
                                if kind == "causal":
                                    f0 = max(0, kc * KCH - qc * QCH)
                                nc.tensor.matmul(
                                    st[:, f0:QCH],
                                    kT[b][h * 64 : (h + 1) * 64, kc * KCH : (kc + 1) * KCH],
                                    qT[b][h * 64 : (h + 1) * 64, qc * QCH + f0 : (qc + 1) * QCH],
                                    start=True,
                                    stop=True,
                                    tile_position=(h * 64, 0),
                                )
                                pt = wpool.tile(
                                    [128, QCH],
                                    f32r,
                                    tag="pt",
                                    bufs=8,
                                    name=f"pt{b}_{qc}_{i}_{h}",
                                )
                                # columns below the causal staircase are entirely
                                # masked: scores/exp/select/PV all skip them; the
                                # final PV must be full width to close the psum
                                # accumulation group, so zero its dead columns
                                if f0 and i == len(kcs) - 1:
                                    nc.gpsimd.tensor_copy(pt[:, 0:f0], zero384[:, 0:f0])
                                nc.scalar.activation(pt[:, f0:QCH], st[:, f0:QCH], AF.Exp)
                                if kind == "causal":
                                    nc.gpsimd.affine_select(
                                        out=pt[:, f0:QCH],
                                        in_=pt[:, f0:QCH],
                                        compare_op=ALU.is_ge,
                                        fill=0.0,
                                        base=qc * QCH - kc * KCH + f0,
                                        pattern=[[1, QCH - f0]],
                                        channel_multiplier=-1,
                                    )
                                elif kind == "mixed":
                                    mt = wpool.tile(
                                        [128, QCH],
                                        f32r,
                                        tag="mt",
                                        bufs=4,
                                        name=f"mt{b}_{qc}_{i}_{h}",
                                    )
                                    nc.sync.dma_start(
                                        mt[:], mb_d[midx * 128 : (midx + 1) * 128, :]
                                    )
                                    nc.vector.tensor_mul(pt[:], pt[:], mt[:])
                                pts[(i, h)] = (pt, f0)

                        def emit_pv(i, b=b, qc=qc, kcs=kcs, pts=pts, acc=acc):
                            kc = kcs[i]
                            for h in range(HPC):
                                pt, f0 = pts.pop((i, h))
                                if i == len(kcs) - 1:
                                    f0 = 0  # full width to close the accumulation group
                                nc.tensor.matmul(
                                    acc[h][0:65, f0:QCH],
                                    vaug[b][h][:, kc * 65 : (kc + 1) * 65],
                                    pt[:, f0:QCH],
                                    start=(i == 0),
                                    stop=(i == len(kcs) - 1),
                                    skip_group_check=(f0 > 0),
                                )

                        ensure_queued(req + 1)
                        for i in range(len(kcs)):
                            emit_scores(i)
                            if i == 1:
                                need_A(req, 5)  # vaug must precede first PV
                            if i == min(1, len(kcs) - 1):
                                flush_pending()
                            if i >= PIPE:
                                emit_pv(i - PIPE)
                            if i >= 3 and i % 2 == 1:
                                pop_fill()
                        need_A(req, 5)
                        for i in range(max(0, len(kcs) - PIPE), len(kcs)):
                            emit_pv(i)

                        # attnT (unnormalized) + reciprocal rows
                        attnT = wpool.tile(
                            [128, QCH], f32r, tag="attnT", bufs=2, name=f"at{b}_{qc}"
                        )
                        recs = []
                        for h in range(HPC):
                            nc.vector.tensor_copy(
                                attnT[h * 64 : (h + 1) * 64, :], acc[h][0:64, :]
                            )
                            rec = wpool.tile(
                                [1, QCH], f32, tag=f"rec{h}", bufs=2, name=f"rec{b}{qc}{h}"
                            )
                            nc.vector.reciprocal(rec[:], acc[h][64:65, :])
                            recr = wpool.tile(
                                [1, QCH], f32r, tag=f"recr{h}", bufs=2, name=f"rr{b}{qc}{h}"
                            )
                            nc.vector.tensor_copy(recr[:], rec[:])
                            recs.append(recr)

                        _last = b == B - 1 and qc == NQC - 1

                        def emit_phase_c(b=b, qc=qc, attnT=attnT, recs=recs, _last=_last):
                            # broadcast 1/denom rows across partitions, normalize attnT
                            for h in range(HPC):
                                bc = ppool.tile(
                                    [128, QCH], f32, tag="acc", bufs=2, name=f"bc{b}_{qc}{h}"
                                )
                                nc.tensor.matmul(
                                    bc[0:64, :],
                                    ones64[:],
                                    recs[h][:],
                                    start=True,
                                    stop=True,
                                )
                                nc.vector.tensor_mul(
                                    attnT[h * 64 : (h + 1) * 64, :],
                                    attnT[h * 64 : (h + 1) * 64, :],
                                    bc[0:64, :],
                                )
                            # output projection: one K=128 matmul per chunk (heads sum)
                            for tk in range(QCH // 128):
                                for oc in range(D // QCH):
                                    op = ppool.tile(
                                        [128, QCH],
                                        f32,
                                        tag="st" if _last else "op",
                                        bufs=5 if _last else 1,
                                        name=f"op{b}_{qc}_{tk}_{oc}",
                                    )
                                    nc.tensor.matmul(
                                        op[:],
                                        attnT[:, tk * 128 : (tk + 1) * 128],
                                        wot[:, oc * QCH : (oc + 1) * QCH],
                                        start=True,
                                        stop=True,
                                    )
                                    osb = wpool.tile(
                                        [128, QCH],
                                        f32,
                                        tag="osb",
                                        bufs=3,
                                        name=f"ob{b}_{qc}_{tk}_{oc}",
                                    )
                                    if (tk + oc) % 2:
                                        nc.scalar.copy(osb[:], op[:])
                                    else:
                                        nc.vector.tensor_copy(osb[:], op[:])
                                    row0 = b * S + qc * QCH + tk * 128
                                    nc.sync.dma_start(
                                        out_d[row0 : row0 + 128, oc * QCH : (oc + 1) * QCH],
                                        osb[:],
                                    )

                        pending.append(emit_phase_c)
                flush_pending()

    nc.compile()
    return nc, blocks, n_mixed


_CACHE = {}


def _get_program(mask):
    key = mask.tobytes()
    if key not in _CACHE:
        _CACHE[key] = _build(mask)
    return _CACHE[key]


def kernel(x, mask, wq, bq, wk, bk, wv, bv, wo, bo):
    x = np.asarray(x, dtype=np.float32)
    mask2 = np.asarray(mask).reshape(S, S)
    nc, blocks, n_mixed = _get_program(mask2)

    xT = np.ascontiguousarray(x.reshape(T, D).T)
    ident = np.ascontiguousarray(np.tile(np.eye(64, dtype=np.float32), (2, 1)))

    if n_mixed:
        mb = np.zeros((n_mixed * 128, QCH), dtype=np.float32)
        for (qc, kc), (kind, midx) in blocks.items():
            if kind == "mixed":
                reg = mask2[qc * QCH : (qc + 1) * QCH, kc * KCH : (kc + 1) * KCH]
                mb[midx * 128 : (midx + 1) * 128, :] = reg.T.astype(np.float32)

    in_maps = []
    for c in range(NCORES):
        hsl = slice(c * HPC * DH, (c + 1) * HPC * DH)
        wqkv = np.concatenate(
            [
                np.asarray(wq)[hsl, :].T / np.sqrt(DH),
                np.asarray(wk)[hsl, :].T,
                np.asarray(wv)[hsl, :].T,
            ],
            axis=1,
        ).astype(np.float32)
        bqkv = np.stack(
            [
                np.asarray(bq)[hsl] / np.sqrt(DH),
                np.asarray(bk)[hsl],
                np.asarray(bv)[hsl],
            ],
            axis=1,
        ).astype(np.float32)
        m = {
            "xt": xT,
            "wqkv": np.ascontiguousarray(wqkv),
            "bqkv": np.ascontiguousarray(bqkv),
            "wot": np.ascontiguousarray(np.asarray(wo)[:, hsl].T.astype(np.float32)),
            "ident": ident,
        }
        if n_mixed:
            m["mblk"] = mb
        in_maps.append(m)

    res = run_bass_kernel_spmd(nc, in_maps, core_ids=list(range(NCORES)))
    out = res.results[0]["out"].astype(np.float64)
    for c in range(1, NCORES):
        out += res.results[c]["out"]
    out = (out + np.asarray(bo)).astype(np.float32)
    return out.reshape(B, S, D)



# revision 4
# speedup vs baseline: 1.0021x; 1.0021x over previous
"""Multi-head attention (B=2, S=2048, D=1024, H=16, causal) on 8 TRN2 cores,
head-parallel: each core computes 2 heads' q/k/v + attention and a partial
output projection; host sums the 8 partials and adds bo.

v2 (bf16): all matmul operands are bf16 (cost-model rate 1 cycle/row at any
moving size; rel-err budget 2e-2 >> bf16 error). HBM traffic is halved and
packed into a handful of large contiguous DMAs via host-side layout:

  xp   (128, 8*8*512)  x^T packed chunk-major: col = t*4096 + dc*512 + c
  wqkv (128, 8*384)    per-dc blocks [wq/sqrt(dh) | wk | wv] columns
  out  (128, 32*1024)  row-block-major: col = blk*1024 + d, blk = token//128

V is projected in flipped orientation (stationary = x sub-chunk, moving = wv
columns) so it lands directly in the [key, dim] layout attention needs - no
PE transposes. Its bias is folded in as a K=1 ones-row matmul. Scores are
computed transposed (S^T[k, q]) per head into halves of one [128,1024] psum
tile so exp / causal-select run once per k-block pair. Normalization: 1/denom
rows (from a ones-column in vaug) broadcast via a tiny ones-matmul, one fused
multiply per chunk; the k-block order puts a full-width block first (psum
start covers every column) and full-width last where possible (clean stop).
"""

import numpy as np
import ml_dtypes

import concourse.bass as bass
import concourse.tile as tile
from concourse import bacc, mybir
from concourse.bass_utils import run_bass_kernel_spmd

B, S, D, H = 2, 2048, 1024, 16
DH = D // H  # 64
NCORES = 8
HPC = H // NCORES  # 2 heads per core
T = B * S  # 4096
QCH = 512
KCH = 128
NQC = S // QCH  # 4
NKC = S // KCH  # 16
NTC = T // QCH  # 8
ND = D // 128  # 8
XC = ND * QCH  # 4096 packed-x columns per token chunk
VW = 65  # vaug block width (64 dims + ones column)

f32 = mybir.dt.float32
bf16 = mybir.dt.bfloat16
AF = mybir.ActivationFunctionType
ALU = mybir.AluOpType
BF = ml_dtypes.bfloat16

PIPE = 3  # score->exp->PV pipeline depth in k-blocks


def _classify_blocks(mask):
    """mask: (S, S) bool [q, k] -> dict (qc, kc) -> (kind, mixed_idx)."""
    blocks = {}
    qg, kg = np.meshgrid(np.arange(S), np.arange(S), indexing="ij")
    causal = qg >= kg
    n_mixed = 0
    for qc in range(NQC):
        for kc in range(NKC):
            reg = mask[qc * QCH : (qc + 1) * QCH, kc * KCH : (kc + 1) * KCH]
            if not reg.any():
                blocks[(qc, kc)] = ("none", -1)
            elif reg.all():
                blocks[(qc, kc)] = ("all", -1)
            elif np.array_equal(
                reg, causal[qc * QCH : (qc + 1) * QCH, kc * KCH : (kc + 1) * KCH]
            ):
                blocks[(qc, kc)] = ("causal", -1)
            else:
                blocks[(qc, kc)] = ("mixed", n_mixed)
                n_mixed += 1
    return blocks, n_mixed


def _order_kcs(blocks, qc):
    """k-block emission order: a full-width block first (its psum write starts
    every column), full-width blocks in the middle, and when possible a
    full-width block last (clean accumulation-group stop)."""
    kcs = [kc for kc in range(NKC) if blocks[(qc, kc)][0] != "none"]
    if not kcs:
        return []

    def f0_of(kc):
        kind, _ = blocks[(qc, kc)]
        return max(0, kc * KCH - qc * QCH) if kind == "causal" else 0

    full = [kc for kc in kcs if f0_of(kc) == 0]
    trimmed = sorted((kc for kc in kcs if f0_of(kc) > 0), key=f0_of, reverse=True)
    assert full, f"q-chunk {qc} has no full-width block"
    if len(full) == 1:
        return [full[0]] + trimmed  # sloppy stop (skip_group_check)
    return full[:-1] + trimmed + [full[-1]]


def _build(mask, reps=1):
    blocks, n_mixed = _classify_blocks(mask)

    nc = bacc.Bacc("TRN2", target_bir_lowering=False, debug=False, num_devices=NCORES)
    x_d = nc.dram_tensor("xp", (128, NTC * XC), bf16, kind="ExternalInput").ap()
    w_d = nc.dram_tensor("wqkv", (128, ND * 384), bf16, kind="ExternalInput").ap()
    bqk_d = nc.dram_tensor("bqk", (128, 2), f32, kind="ExternalInput").ap()
    bvo_d = nc.dram_tensor("bvo", (1, 128), bf16, kind="ExternalInput").ap()
    wo_d = nc.dram_tensor("wot", (128, D), bf16, kind="ExternalInput").ap()
    out_d = nc.dram_tensor("out", (128, (T // 128) * D), bf16, kind="ExternalOutput").ap()
    if n_mixed:
        mb_d = nc.dram_tensor("mblk", (n_mixed * 128, QCH), bf16, kind="ExternalInput").ap()

    def pair_ap(t, f0, width):
        """[128, (2 heads, width)] view of a [128, 1024] tile at column f0."""
        return bass.AP(t.tensor, t.offset + f0, [t.ap[0], [512, 2], [1, width]])

    with tile.TileContext(nc) as tc:
        with (
            tc.tile_pool(name="const", bufs=1) as cpool,
            tc.tile_pool(name="work", bufs=1) as wpool,
            tc.tile_pool(name="psum", bufs=1, space="PSUM") as ppool,
        ):
            # ---- input stream: interleave weights and early x so the first
            # projection matmuls start as soon as (w-dc0/1, x-chunk0) land ----
            w = cpool.tile([128, ND * 384], bf16, name="w")
            xall = cpool.tile([128, NTC * XC], bf16, name="xall")
            nc.sync.dma_start(w[:, 0:768], w_d[:, 0:768])
            nc.sync.dma_start(xall[:, 0:1024], x_d[:, 0:1024])
            nc.sync.dma_start(w[:, 768:3072], w_d[:, 768:3072])
            nc.sync.dma_start(xall[:, 1024:4096], x_d[:, 1024:4096])
            bqk = cpool.tile([128, 2], f32, name="bqk")
            nc.sync.dma_start(bqk[:], bqk_d)
            bvo = cpool.tile([1, 128], bf16, name="bvo")
            nc.sync.dma_start(bvo[:], bvo_d)
            wot = cpool.tile([128, D], bf16, name="wot")
            nc.sync.dma_start(wot[:], wo_d)
            for t in range(1, NTC):
                nc.sync.dma_start(
                    xall[:, t * XC : (t + 1) * XC], x_d[:, t * XC : (t + 1) * XC]
                )

            ones1 = cpool.tile([1, 128], bf16, name="ones1")
            nc.vector.memset(ones1[:], 1.0)
            ones64 = cpool.tile([1, 64], bf16, name="ones64")
            nc.vector.memset(ones64[:], 1.0)

            # ---- per-batch persistent activations ----
            qT = [cpool.tile([128, S], bf16, name=f"qT{b}") for b in range(B)]
            kT = [cpool.tile([128, S], bf16, name=f"kT{b}") for b in range(B)]
            # vaug[b]: h-major [128 keys, 2 * 16 * 65]; col 64 of each
            # 65-block is the ones column producing softmax denominators
            vaug = [cpool.tile([128, HPC * NKC * VW], bf16, name=f"va{b}") for b in range(B)]
            for b in range(B):
                nc.vector.memset(vaug[b][:, 64::VW], 1.0)

            def vslice(b, h, kc):
                return vaug[b][:, h * NKC * VW + kc * VW : h * NKC * VW + kc * VW + VW]

            for _rep in range(reps):
                # ---- phase A units (drained on demand into the k-loops) ----
                def unit_qk(t):
                    b, tq = t // NQC, t % NQC
                    ps = ppool.tile([128, 1024], f32, tag="st", bufs=3, name=f"ps{t}")
                    for p in range(2):
                        for dc in range(ND):
                            nc.tensor.matmul(
                                ps[:, p * 512 : (p + 1) * 512],
                                w[:, dc * 384 + p * 128 : dc * 384 + (p + 1) * 128],
                                xall[:, t * XC + dc * 512 : t * XC + (dc + 1) * 512],
                                start=(dc == 0),
                                stop=(dc == ND - 1),
                            )
                    nc.scalar.activation(
                        qT[b][:, tq * 512 : (tq + 1) * 512],
                        ps[:, 0:512],
                        AF.Identity,
                        bias=bqk[:, 0:1],
                    )
                    nc.vector.tensor_scalar_add(
                        kT[b][:, tq * 512 : (tq + 1) * 512],
                        ps[:, 512:1024],
                        bqk[:, 1:2],
                    )

                vps_cell = {}

                def unit_v(t):
                    vp = ppool.tile([128, 1024], f32, tag="st", bufs=3, name=f"vp{t}")
                    for j in range(4):
                        for dc in range(ND):
                            nc.tensor.matmul(
                                vp[:, j * 128 : (j + 1) * 128],
                                xall[:, t * XC + dc * 512 + j * 128 : t * XC + dc * 512 + (j + 1) * 128],
                                w[:, dc * 384 + 256 : dc * 384 + 384],
                                start=(dc == 0),
                                stop=False,
                            )
                        nc.tensor.matmul(
                            vp[:, j * 128 : (j + 1) * 128],
                            ones1[:],
                            bvo[:],
                            start=False,
                            stop=True,
                        )
                    vps_cell[t] = vp

                def unit_vcopy(t):
                    b, tq = t // NQC, t % NQC
                    vp = vps_cell.pop(t)
                    va = vaug[b]
                    dst = bass.AP(
                        va.tensor,
                        va.offset + tq * 4 * VW,
                        [va.ap[0], [NKC * VW, 2], [VW, 4], [1, 64]],
                    )
                    src = bass.AP(
                        vp.tensor, vp.offset, [vp.ap[0], [64, 2], [128, 4], [1, 64]]
                    )
                    nc.vector.tensor_copy(dst, src)

                fill = []  # (chunk, unit_idx, closure)

                def queue_A(t):
                    if t >= NTC:
                        return
                    fill.append((t, 0, lambda t=t: unit_qk(t)))
                    fill.append((t, 1, lambda t=t: unit_v(t)))
                    fill.append((t, 2, lambda t=t: unit_vcopy(t)))

                def need_A(t, n_units):
                    while fill and (
                        fill[0][0] < t or (fill[0][0] == t and fill[0][1] < n_units)
                    ):
                        fill.pop(0)[2]()

                def pop_fill():
                    if fill:
                        fill.pop(0)[2]()

                queued = [0]

                def ensure_queued(t):
                    while queued[0] <= min(t, NTC - 1):
                        queue_A(queued[0])
                        queued[0] += 1

                # ---- phases B/C per (batch, q-chunk) ----
                pending = []

                def flush_pending():
                    while pending:
                        pending.pop(0)()

                for b in range(B):
                    for qc in range(NQC):
                        kcs = _order_kcs(blocks, qc)
                        kmax = max(kcs) // (QCH // KCH) if kcs else 0
                        req = b * NQC + max(qc, kmax)
                        ensure_queued(req)
                        need_A(req, 1)  # qT/kT of this chunk before scores
                        # first k-block index (emission order) whose PV reads
                        # vaug written by this chunk's own phase A
                        first_own = min(
                            (i for i, kc in enumerate(kcs) if kc // (QCH // KCH) >= qc),
                            default=len(kcs),
                        )
                        acc = ppool.tile(
                            [128, 1024], f32, tag="acc", bufs=1, name=f"acc{b}_{qc}"
                        )
                        pts = {}
                        sloppy_stop = blocks[(qc, kcs[-1])][0] == "causal" and (
                            kcs[-1] * KCH > qc * QCH
                        )

                        def emit_scores(i, b=b, qc=qc, kcs=kcs, pts=pts):
                            kc = kcs[i]
                            kind, midx = blocks[(qc, kc)]
                            f0 = 0
                            if kind == "causal":
                                f0 = max(0, kc * KCH - qc * QCH)
                            st = ppool.tile(
                                [128, 1024], f32, tag="st", bufs=3, name=f"st{b}_{qc}_{i}"
                            )
                            for h in range(HPC):
                                nc.tensor.matmul(
                                    st[:, h * 512 + f0 : (h + 1) * 512],
                                    kT[b][h * 64 : (h + 1) * 64, kc * KCH : (kc + 1) * KCH],
                                    qT[b][h * 64 : (h + 1) * 64, qc * QCH + f0 : (qc + 1) * QCH],
                                    start=True,
                                    stop=True,
                                    tile_position=(h * 64, 0),
                                )
                            pt = wpool.tile(
                                [128, 1024], bf16, tag="pt", bufs=6, name=f"pt{b}_{qc}_{i}"
                            )
                            if f0:
                                nc.scalar.activation(
                                    pair_ap(pt, f0, 512 - f0), pair_ap(st, f0, 512 - f0), AF.Exp
                                )
                            else:
                                nc.scalar.activation(pt[:], st[:], AF.Exp)
                            if kind == "causal":
                                nc.gpsimd.affine_select(
                                    out=pair_ap(pt, f0, 512 - f0),
                                    in_=pair_ap(pt, f0, 512 - f0),
                                    compare_op=ALU.is_ge,
                                    fill=0.0,
                                    base=qc * QCH - kc * KCH + f0,
                                    pattern=[[0, 2], [1, 512 - f0]],
                                    channel_multiplier=-1,
                                )
                            elif kind == "mixed":
                                mt = wpool.tile(
                                    [128, QCH], bf16, tag="mt", bufs=4, name=f"mt{b}_{qc}_{i}"
                                )
                                nc.sync.dma_start(mt[:], mb_d[midx * 128 : (midx + 1) * 128, :])
                                for h in range(HPC):
                                    nc.vector.tensor_mul(
                                        pt[:, h * 512 : (h + 1) * 512],
                                        pt[:, h * 512 : (h + 1) * 512],
                                        mt[:],
                                    )
                            pts[(i,)] = (pt, f0)

                        def emit_pv(i, b=b, qc=qc, kcs=kcs, pts=pts, acc=acc,
                                    sloppy=sloppy_stop, first_own=first_own, req=req):
                            if i >= first_own:
                                need_A(req, 3)  # vaug of this chunk before own-PV
                            kc = kcs[i]
                            pt, f0 = pts.pop((i,))
                            last = i == len(kcs) - 1
                            for h in range(HPC):
                                nc.tensor.matmul(
                                    acc[0:65, h * 512 + f0 : (h + 1) * 512],
                                    vslice(b, h, kc),
                                    pt[:, h * 512 + f0 : (h + 1) * 512],
                                    start=(i == 0),
                                    stop=last,
                                    skip_group_check=(f0 > 0 or (last and sloppy)),
                                )

                        ensure_queued(req + 1)
                        for i in range(len(kcs)):
                            emit_scores(i)
                            if i == min(1, len(kcs) - 1):
                                flush_pending()
                            if i >= PIPE:
                                emit_pv(i - PIPE)
                            if i >= 3 and i % 2 == 1:
                                pop_fill()
                        for i in range(max(0, len(kcs) - PIPE), len(kcs)):
                            emit_pv(i)

                        # ---- phase C: reciprocal + unnormalized attnT now;
                        # broadcast/normalize/out-proj deferred into the next
                        # chunk's k-loop ----
                        rec = wpool.tile([1, 1024], bf16, tag="rec", bufs=2, name=f"rc{b}{qc}")
                        with nc.allow_low_precision(reason="softmax 1/denom in bf16"):
                            nc.vector.reciprocal(
                                rec[:],
                                bass.AP(
                                    acc.tensor,
                                    acc.offset + 64 * acc.ap[0][0],
                                    [[acc.ap[0][0], 1], [1, 1024]],
                                ),
                            )
                        attnT = wpool.tile(
                            [128, QCH], bf16, tag="attnT", bufs=2, name=f"at{b}_{qc}"
                        )
                        for h in range(HPC):
                            nc.scalar.copy(
                                attnT[h * 64 : (h + 1) * 64, :],
                                acc[0:64, h * 512 : (h + 1) * 512],
                            )

                        _last = b == B - 1 and qc == NQC - 1

                        def emit_phase_c(b=b, qc=qc, attnT=attnT, rec=rec, _last=_last):
                            bc = ppool.tile(
                                [128, 1024], f32, tag="st", bufs=3, name=f"bc{b}_{qc}"
                            )
                            for h in range(HPC):
                                nc.tensor.matmul(
                                    bc[h * 64 : (h + 1) * 64, 0:512],
                                    ones64[:],
                                    rec[0:1, h * 512 : (h + 1) * 512],
                                    start=True,
                                    stop=True,
                                    tile_position=(0, h * 64),
                                )
                            nc.vector.tensor_mul(attnT[:], attnT[:], bc[:, 0:512])
                            osb = wpool.tile(
                                [128, 4096], bf16, tag="osb", bufs=2, name=f"ob{b}_{qc}"
                            )
                            blk0 = (b * S + qc * QCH) // 128
                            for tk in range(4):
                                op = ppool.tile(
                                    [128, 1024], f32, tag="st", bufs=3, name=f"op{b}_{qc}_{tk}"
                                )
                                for oc in range(2):
                                    nc.tensor.matmul(
                                        op[:, oc * 512 : (oc + 1) * 512],
                                        attnT[:, tk * 128 : (tk + 1) * 128],
                                        wot[:, oc * 512 : (oc + 1) * 512],
                                        start=True,
                                        stop=True,
                                    )
                                if tk % 2:
                                    nc.scalar.copy(osb[:, tk * 1024 : (tk + 1) * 1024], op[:])
                                else:
                                    nc.vector.tensor_copy(
                                        osb[:, tk * 1024 : (tk + 1) * 1024], op[:]
                                    )
                                if _last:
                                    nc.sync.dma_start(
                                        out_d[:, (blk0 + tk) * 1024 : (blk0 + tk + 1) * 1024],
                                        osb[:, tk * 1024 : (tk + 1) * 1024],
                                    )
                            if not _last:
                                nc.sync.dma_start(
                                    out_d[:, blk0 * 1024 : (blk0 + 4) * 1024], osb[:]
                                )

                        pending.append(emit_phase_c)
                flush_pending()

    nc.compile()
    return nc, blocks, n_mixed


_CACHE = {}


def _get_program(mask):
    key = mask.tobytes()
    if key not in _CACHE:
        _CACHE[key] = _build(mask)
    return _CACHE[key]


def kernel(x, mask, wq, bq, wk, bk, wv, bv, wo, bo):
    x = np.asarray(x, dtype=np.float32)
    mask2 = np.asarray(mask).reshape(S, S)
    nc, blocks, n_mixed = _get_program(mask2)

    # pack x^T chunk-major: xp[p, t*4096 + dc*512 + c] = x[token t*512+c, dc*128+p]
    xp = np.ascontiguousarray(
        x.reshape(NTC, QCH, ND, 128).transpose(3, 0, 2, 1).reshape(128, NTC * XC)
    ).astype(BF)

    if n_mixed:
        mb = np.zeros((n_mixed * 128, QCH), dtype=BF)
        for (qc, kc), (kind, midx) in blocks.items():
            if kind == "mixed":
                reg = mask2[qc * QCH : (qc + 1) * QCH, kc * KCH : (kc + 1) * KCH]
                mb[midx * 128 : (midx + 1) * 128, :] = reg.T.astype(BF)

    scale = 1.0 / np.sqrt(DH)
    in_maps = []
    for c in range(NCORES):
        hsl = slice(c * HPC * DH, (c + 1) * HPC * DH)
        wq_c = np.asarray(wq)[hsl, :].T * scale  # (1024, 128)
        wk_c = np.asarray(wk)[hsl, :].T
        wv_c = np.asarray(wv)[hsl, :].T
        wqkv = np.concatenate(
            [
                np.stack([wq_c[dc * 128 : (dc + 1) * 128] for dc in range(ND)]),
                np.stack([wk_c[dc * 128 : (dc + 1) * 128] for dc in range(ND)]),
                np.stack([wv_c[dc * 128 : (dc + 1) * 128] for dc in range(ND)]),
            ],
            axis=2,
        )  # (ND, 128, 384)
        m = {
            "xp": xp,
            "wqkv": np.ascontiguousarray(
                wqkv.transpose(1, 0, 2).reshape(128, ND * 384)
            ).astype(BF),
            "bqk": np.ascontiguousarray(
                np.stack([np.asarray(bq)[hsl] * scale, np.asarray(bk)[hsl]], axis=1)
            ).astype(np.float32),
            "bvo": np.asarray(bv)[hsl].reshape(1, 128).astype(BF),
            "wot": np.ascontiguousarray(np.asarray(wo)[:, hsl].T).astype(BF),
        }
        if n_mixed:
            m["mblk"] = mb
        in_maps.append(m)

    res = run_bass_kernel_spmd(nc, in_maps, core_ids=list(range(NCORES)))
    out = np.zeros((128, (T // 128) * D), dtype=np.float64)
    for c in range(NCORES):
        out += res.results[c]["out"].astype(np.float64)
    # unpack row-block-major (128, 32*1024) -> (T, D)
    out = out.reshape(128, T // 128, D).transpose(1, 0, 2).reshape(T, D)
    out = (out + np.asarray(bo)).astype(np.float32)
    return out.reshape(B, S, D)


# revision 11
# speedup vs baseline: 1.1208x; 1.1184x over previous
"""Multi-head attention (B=2, S=2048, D=1024, H=16, causal) on 8 TRN2 cores,
head-parallel: each core computes 2 heads' q/k/v + attention and a partial
output projection; host sums the 8 partials and adds bo.

v2 (bf16): all matmul operands are bf16 (cost-model rate 1 cycle/row at any
moving size; rel-err budget 2e-2 >> bf16 error). HBM traffic is halved and
packed into a handful of large contiguous DMAs via host-side layout:

  xp   (128, 8*8*512)  x^T packed chunk-major: col = t*4096 + dc*512 + c
  wqkv (128, 8*384)    per-dc blocks [wq/sqrt(dh) | wk | wv] columns
  out  (128, 32*1024)  row-block-major: col = blk*1024 + d, blk = token//128

V is projected in flipped orientation (stationary = x sub-chunk, moving = wv
columns) so it lands directly in the [key, dim] layout attention needs - no
PE transposes. Its bias is folded in as a K=1 ones-row matmul. Scores are
computed transposed (S^T[k, q]) per head into halves of one [128,1024] psum
tile so exp / causal-select run once per k-block pair. Normalization: 1/denom
rows (from a ones-column in vaug) broadcast via a tiny ones-matmul, one fused
multiply per chunk; the k-block order puts a full-width block first (psum
start covers every column) and full-width last where possible (clean stop).
"""

import numpy as np
import ml_dtypes

import concourse.bass as bass
import concourse.tile as tile
from concourse import bacc, mybir
from concourse.bass_utils import run_bass_kernel_spmd

B, S, D, H = 2, 2048, 1024, 16
DH = D // H  # 64
NCORES = 8
HPC = H // NCORES  # 2 heads per core
T = B * S  # 4096
QCH = 512
KCH = 128
NQC = S // QCH  # 4
NKC = S // KCH  # 16
NTC = T // QCH  # 8
ND = D // 128  # 8
XC = ND * QCH  # 4096 packed-x columns per token chunk
VW = 65  # vaug block width (64 dims + ones column)

f32 = mybir.dt.float32
bf16 = mybir.dt.bfloat16
AF = mybir.ActivationFunctionType
ALU = mybir.AluOpType
BF = ml_dtypes.bfloat16

PIPE = 4  # score->exp->PV pipeline depth in k-blocks


def _classify_blocks(mask):
    """mask: (S, S) bool [q, k] -> dict (qc, kc) -> (kind, mixed_idx)."""
    blocks = {}
    qg, kg = np.meshgrid(np.arange(S), np.arange(S), indexing="ij")
    causal = qg >= kg
    n_mixed = 0
    for qc in range(NQC):
        for kc in range(NKC):
            reg = mask[qc * QCH : (qc + 1) * QCH, kc * KCH : (kc + 1) * KCH]
            if not reg.any():
                blocks[(qc, kc)] = ("none", -1)
            elif reg.all():
                blocks[(qc, kc)] = ("all", -1)
            elif np.array_equal(
                reg, causal[qc * QCH : (qc + 1) * QCH, kc * KCH : (kc + 1) * KCH]
            ):
                blocks[(qc, kc)] = ("causal", -1)
            else:
                blocks[(qc, kc)] = ("mixed", n_mixed)
                n_mixed += 1
    return blocks, n_mixed


def _order_kcs(blocks, qc):
    """k-block emission order: a full-width block first (its psum write starts
    every column), full-width blocks in the middle, and when possible a
    full-width block last (clean accumulation-group stop)."""
    kcs = [kc for kc in range(NKC) if blocks[(qc, kc)][0] != "none"]
    if not kcs:
        return []

    def f0_of(kc):
        kind, _ = blocks[(qc, kc)]
        return max(0, kc * KCH - qc * QCH) if kind == "causal" else 0

    full = [kc for kc in kcs if f0_of(kc) == 0]
    trimmed = sorted((kc for kc in kcs if f0_of(kc) > 0), key=f0_of, reverse=True)
    assert full, f"q-chunk {qc} has no full-width block"
    if len(full) == 1:
        return [full[0]] + trimmed  # sloppy stop (skip_group_check)
    return full[:-1] + trimmed + [full[-1]]


def _build(mask, reps=1):
    blocks, n_mixed = _classify_blocks(mask)

    nc = bacc.Bacc("TRN2", target_bir_lowering=False, debug=False, num_devices=NCORES)
    x_d = nc.dram_tensor("xp", (128, NTC * XC), bf16, kind="ExternalInput").ap()
    w_d = nc.dram_tensor("wqkv", (128, ND * 384), bf16, kind="ExternalInput").ap()
    bqk_d = nc.dram_tensor("bqk", (128, 2), f32, kind="ExternalInput").ap()
    bvo_d = nc.dram_tensor("bvo", (1, 128), bf16, kind="ExternalInput").ap()
    wo_d = nc.dram_tensor("wot", (128, D), bf16, kind="ExternalInput").ap()
    out_d = nc.dram_tensor("out", (128, (T // 128) * D), bf16, kind="ExternalOutput").ap()
    if n_mixed:
        mb_d = nc.dram_tensor("mblk", (n_mixed * 128, QCH), bf16, kind="ExternalInput").ap()

    def pair_ap(t, f0, width):
        """[128, (2 heads, width)] view of a [128, 1024] tile at column f0."""
        return bass.AP(t.tensor, t.offset + f0, [t.ap[0], [512, 2], [1, width]])

    with tile.TileContext(nc) as tc:
        with (
            tc.tile_pool(name="const", bufs=1) as cpool,
            tc.tile_pool(name="work", bufs=1) as wpool,
            tc.tile_pool(name="psum", bufs=1, space="PSUM") as ppool,
        ):
            # ---- input stream: interleave weights and early x so the first
            # projection matmuls start as soon as (w-dc0/1, x-chunk0) land ----
            w = cpool.tile([128, ND * 384], bf16, name="w")
            xall = cpool.tile([128, NTC * XC], bf16, name="xall")
            nc.sync.dma_start(w[:, 0:768], w_d[:, 0:768])
            nc.sync.dma_start(xall[:, 0:1024], x_d[:, 0:1024])
            nc.sync.dma_start(w[:, 768:3072], w_d[:, 768:3072])
            nc.sync.dma_start(xall[:, 1024:4096], x_d[:, 1024:4096])
            bqk = cpool.tile([128, 2], f32, name="bqk")
            nc.sync.dma_start(bqk[:], bqk_d)
            bvo = cpool.tile([1, 128], bf16, name="bvo")
            nc.sync.dma_start(bvo[:], bvo_d)
            wot = cpool.tile([128, D], bf16, name="wot")
            nc.sync.dma_start(wot[:], wo_d)
            for t in range(1, NTC):
                nc.sync.dma_start(
                    xall[:, t * XC : (t + 1) * XC], x_d[:, t * XC : (t + 1) * XC]
                )

            ones1 = cpool.tile([1, 128], bf16, name="ones1")
            nc.vector.memset(ones1[:], 1.0)
            ones64 = cpool.tile([1, 64], bf16, name="ones64")
            nc.vector.memset(ones64[:], 1.0)

            # ---- per-batch persistent activations ----
            qT = [cpool.tile([128, S], bf16, name=f"qT{b}") for b in range(B)]
            kT = [cpool.tile([128, S], bf16, name=f"kT{b}") for b in range(B)]
            # vaug[b]: h-major [128 keys, 2 * 16 * 65]; col 64 of each
            # 65-block is the ones column producing softmax denominators
            vaug = [cpool.tile([128, HPC * NKC * VW], bf16, name=f"va{b}") for b in range(B)]
            for b in range(B):
                nc.vector.memset(vaug[b][:, 64::VW], 1.0)

            def vslice(b, h, kc):
                return vaug[b][:, h * NKC * VW + kc * VW : h * NKC * VW + kc * VW + VW]

            for _rep in range(reps):
                # ---- phase A units (fine-grained, drained into the k-loops
                # a sub-microsecond piece at a time so the score->exp->PV
                # pipeline never sees a burst of projection matmuls) ----
                cells = {}  # t -> (ps, vp)

                def unit_qk_mm(t, p, half):
                    b, tq = t // NQC, t % NQC
                    if p == 0 and half == 0:
                        cells[t] = [
                            ppool.tile([128, 1024], f32, tag="st", bufs=3, name=f"ps{t}"),
                            None,
                        ]
                    ps = cells[t][0]
                    for dc in range(half * 4, half * 4 + 4):
                        nc.tensor.matmul(
                            ps[:, p * 512 : (p + 1) * 512],
                            w[:, dc * 384 + p * 128 : dc * 384 + (p + 1) * 128],
                            xall[:, t * XC + dc * 512 : t * XC + (dc + 1) * 512],
                            start=(dc == 0),
                            stop=(dc == ND - 1),
                        )

                def unit_qk_moves(t):
                    b, tq = t // NQC, t % NQC
                    ps = cells[t][0]
                    nc.vector.tensor_scalar_add(
                        qT[b][:, tq * 512 : (tq + 1) * 512], ps[:, 0:512], bqk[:, 0:1]
                    )
                    nc.vector.tensor_scalar_add(
                        kT[b][:, tq * 512 : (tq + 1) * 512], ps[:, 512:1024], bqk[:, 1:2]
                    )

                def unit_v_mm(t, j):
                    if j == 0:
                        cells[t][1] = ppool.tile(
                            [128, 1024], f32, tag="st", bufs=3, name=f"vp{t}"
                        )
                    vp = cells[t][1]
                    for dc in range(ND):
                        nc.tensor.matmul(
                            vp[:, j * 128 : (j + 1) * 128],
                            xall[:, t * XC + dc * 512 + j * 128 : t * XC + dc * 512 + (j + 1) * 128],
                            w[:, dc * 384 + 256 : dc * 384 + 384],
                            start=(dc == 0),
                            stop=False,
                        )
                    nc.tensor.matmul(
                        vp[:, j * 128 : (j + 1) * 128], ones1[:], bvo[:],
                        start=False, stop=True,
                    )

                def unit_vcopy(t):
                    b, tq = t // NQC, t % NQC
                    vp = cells.pop(t)[1]
                    va = vaug[b]
                    dst = bass.AP(
                        va.tensor,
                        va.offset + tq * 4 * VW,
                        [va.ap[0], [NKC * VW, 2], [VW, 4], [1, 64]],
                    )
                    src = bass.AP(
                        vp.tensor, vp.offset, [vp.ap[0], [64, 2], [128, 4], [1, 64]]
                    )
                    nc.vector.tensor_copy(dst, src)

                fill = []  # (chunk, unit_idx, closure)
                N_UNITS = 10
                MOVES_DONE = 5  # units < 5: qk matmuls + moves (scores need these)

                def queue_A(t):
                    if t >= NTC:
                        return
                    u = 0
                    for p in range(2):
                        for half in range(2):
                            fill.append((t, u, lambda t=t, p=p, h=half: unit_qk_mm(t, p, h)))
                            u += 1
                    fill.append((t, u, lambda t=t: unit_qk_moves(t)))
                    u += 1
                    for j in range(4):
                        fill.append((t, u, lambda t=t, j=j: unit_v_mm(t, j)))
                        u += 1
                    fill.append((t, u, lambda t=t: unit_vcopy(t)))

                def need_A(t, n_units):
                    while fill and (
                        fill[0][0] < t or (fill[0][0] == t and fill[0][1] < n_units)
                    ):
                        fill.pop(0)[2]()

                def pop_fill():
                    if fill:
                        fill.pop(0)[2]()
                        return True
                    return False

                queued = [0]

                def ensure_queued(t):
                    while queued[0] <= min(t, NTC - 1):
                        queue_A(queued[0])
                        queued[0] += 1

                # ---- phases B/C per (batch, q-chunk) ----
                pending = []

                def pop_pending():
                    if pending:
                        pending.pop(0)()
                        return True
                    return False

                def flush_pending():
                    while pending:
                        pending.pop(0)()

                for b in range(B):
                    for qc in range(NQC):
                        kcs = _order_kcs(blocks, qc)
                        kmax = max(kcs) // (QCH // KCH) if kcs else 0
                        req = b * NQC + max(qc, kmax)
                        ensure_queued(req)
                        need_A(req, MOVES_DONE)  # qT/kT of this chunk before scores
                        # first k-block index (emission order) whose PV reads
                        # vaug written by this chunk's own phase A
                        first_own = min(
                            (i for i, kc in enumerate(kcs) if kc // (QCH // KCH) >= qc),
                            default=len(kcs),
                        )
                        acc = ppool.tile(
                            [128, 1024], f32, tag="acc", bufs=1, name=f"acc{b}_{qc}"
                        )
                        pts = {}
                        sloppy_stop = blocks[(qc, kcs[-1])][0] == "causal" and (
                            kcs[-1] * KCH > qc * QCH
                        )

                        def emit_scores(i, b=b, qc=qc, kcs=kcs, pts=pts):
                            kc = kcs[i]
                            kind, midx = blocks[(qc, kc)]
                            f0 = 0
                            if kind == "causal":
                                f0 = max(0, kc * KCH - qc * QCH)
                            st = ppool.tile(
                                [128, 1024], f32, tag="st", bufs=3, name=f"st{b}_{qc}_{i}"
                            )
                            for h in range(HPC):
                                nc.tensor.matmul(
                                    st[:, h * 512 + f0 : (h + 1) * 512],
                                    kT[b][h * 64 : (h + 1) * 64, kc * KCH : (kc + 1) * KCH],
                                    qT[b][h * 64 : (h + 1) * 64, qc * QCH + f0 : (qc + 1) * QCH],
                                    start=True,
                                    stop=True,
                                    tile_position=(h * 64, 0),
                                )
                            pt = wpool.tile(
                                [128, 1024], bf16, tag="pt", bufs=8, name=f"pt{b}_{qc}_{i}"
                            )
                            if f0:
                                nc.scalar.activation(
                                    pair_ap(pt, f0, 512 - f0), pair_ap(st, f0, 512 - f0), AF.Exp
                                )
                            else:
                                nc.scalar.activation(pt[:], st[:], AF.Exp)
                            if kind == "causal":
                                nc.gpsimd.affine_select(
                                    out=pair_ap(pt, f0, 512 - f0),
                                    in_=pair_ap(pt, f0, 512 - f0),
                                    compare_op=ALU.is_ge,
                                    fill=0.0,
                                    base=qc * QCH - kc * KCH + f0,
                                    pattern=[[0, 2], [1, 512 - f0]],
                                    channel_multiplier=-1,
                                )
                            elif kind == "mixed":
                                mt = wpool.tile(
                                    [128, QCH], bf16, tag="mt", bufs=4, name=f"mt{b}_{qc}_{i}"
                                )
                                nc.sync.dma_start(mt[:], mb_d[midx * 128 : (midx + 1) * 128, :])
                                for h in range(HPC):
                                    nc.vector.tensor_mul(
                                        pt[:, h * 512 : (h + 1) * 512],
                                        pt[:, h * 512 : (h + 1) * 512],
                                        mt[:],
                                    )
                            pts[(i,)] = (pt, f0)

                        def emit_pv(i, b=b, qc=qc, kcs=kcs, pts=pts, acc=acc,
                                    sloppy=sloppy_stop, first_own=first_own, req=req):
                            if i >= first_own:
                                need_A(req, N_UNITS)  # vaug of this chunk before own-PV
                            kc = kcs[i]
                            pt, f0 = pts.pop((i,))
                            last = i == len(kcs) - 1
                            for h in range(HPC):
                                nc.tensor.matmul(
                                    acc[0:65, h * 512 + f0 : (h + 1) * 512],
                                    vslice(b, h, kc),
                                    pt[:, h * 512 + f0 : (h + 1) * 512],
                                    start=(i == 0),
                                    stop=last,
                                    skip_group_check=(f0 > 0 or (last and sloppy)),
                                )

                        ensure_queued(req + 1)
                        for i in range(len(kcs)):
                            emit_scores(i)
                            if i >= PIPE:
                                emit_pv(i - PIPE)
                            # drain next chunk's projections first (their
                            # qT/kT feed the next loop), then prior chunk's
                            # deferred normalize/out-projection
                            if i >= 1:
                                if not (pop_fill() and pop_fill()):
                                    pop_pending()
                        for i in range(max(0, len(kcs) - PIPE), len(kcs)):
                            emit_pv(i)
                            pop_pending()

                        # ---- phase C: reciprocal + unnormalized attnT now;
                        # broadcast/normalize/out-proj deferred into the next
                        # chunk's k-loop as five sub-microsecond pieces ----
                        rec = wpool.tile([1, 1024], bf16, tag="rec", bufs=2, name=f"rc{b}{qc}")
                        with nc.allow_low_precision(reason="softmax 1/denom in bf16"):
                            nc.vector.reciprocal(
                                rec[:],
                                bass.AP(
                                    acc.tensor,
                                    acc.offset + 64 * acc.ap[0][0],
                                    [[acc.ap[0][0], 1], [1, 1024]],
                                ),
                            )
                        attnT = wpool.tile(
                            [128, QCH], bf16, tag="attnT", bufs=2, name=f"at{b}_{qc}"
                        )
                        for h in range(HPC):
                            nc.scalar.copy(
                                attnT[h * 64 : (h + 1) * 64, :],
                                acc[0:64, h * 512 : (h + 1) * 512],
                            )
                        flush_pending()  # any leftover phase C of the prior chunk

                        _last = b == B - 1 and qc == NQC - 1
                        osb = wpool.tile(
                            [128, 4096], bf16, tag="osb", bufs=2, name=f"ob{b}_{qc}"
                        )

                        def sub_norm(b=b, qc=qc, attnT=attnT, rec=rec):
                            bc = ppool.tile(
                                [128, 1024], f32, tag="st", bufs=3, name=f"bc{b}_{qc}"
                            )
                            for h in range(HPC):
                                nc.tensor.matmul(
                                    bc[h * 64 : (h + 1) * 64, 0:512],
                                    ones64[:],
                                    rec[0:1, h * 512 : (h + 1) * 512],
                                    start=True,
                                    stop=True,
                                    tile_position=(0, h * 64),
                                )
                            nc.vector.tensor_mul(attnT[:], attnT[:], bc[:, 0:512])

                        def sub_oproj(tk, b=b, qc=qc, attnT=attnT, osb=osb, _last=_last):
                            blk0 = (b * S + qc * QCH) // 128
                            op = ppool.tile(
                                [128, 1024], f32, tag="st", bufs=3, name=f"op{b}_{qc}_{tk}"
                            )
                            for oc in range(2):
                                nc.tensor.matmul(
                                    op[:, oc * 512 : (oc + 1) * 512],
                                    attnT[:, tk * 128 : (tk + 1) * 128],
                                    wot[:, oc * 512 : (oc + 1) * 512],
                                    start=True,
                                    stop=True,
                                )
                            if tk % 2:
                                nc.scalar.copy(osb[:, tk * 1024 : (tk + 1) * 1024], op[:])
                            else:
                                nc.vector.tensor_copy(
                                    osb[:, tk * 1024 : (tk + 1) * 1024], op[:]
                                )
                            blk0 = (b * S + qc * QCH) // 128
                            if _last:
                                nc.sync.dma_start(
                                    out_d[:, (blk0 + tk) * 1024 : (blk0 + tk + 1) * 1024],
                                    osb[:, tk * 1024 : (tk + 1) * 1024],
                                )
                            elif tk == 3:
                                nc.sync.dma_start(
                                    out_d[:, blk0 * 1024 : (blk0 + 4) * 1024], osb[:]
                                )

                        pending.append(sub_norm)
                        for tk in range(4):
                            pending.append(lambda tk=tk: sub_oproj(tk))
                flush_pending()

    nc.compile()
    return nc, blocks, n_mixed


_CACHE = {}


def _get_program(mask):
    key = mask.tobytes()
    if key not in _CACHE:
        _CACHE[key] = _build(mask)
    return _CACHE[key]


def kernel(x, mask, wq, bq, wk, bk, wv, bv, wo, bo):
    x = np.asarray(x, dtype=np.float32)
    mask2 = np.asarray(mask).reshape(S, S)
    nc, blocks, n_mixed = _get_program(mask2)

    # pack x^T chunk-major: xp[p, t*4096 + dc*512 + c] = x[token t*512+c, dc*128+p]
    xp = np.ascontiguousarray(
        x.reshape(NTC, QCH, ND, 128).transpose(3, 0, 2, 1).reshape(128, NTC * XC)
    ).astype(BF)

    if n_mixed:
        mb = np.zeros((n_mixed * 128, QCH), dtype=BF)
        for (qc, kc), (kind, midx) in blocks.items():
            if kind == "mixed":
                reg = mask2[qc * QCH : (qc + 1) * QCH, kc * KCH : (kc + 1) * KCH]
                mb[midx * 128 : (midx + 1) * 128, :] = reg.T.astype(BF)

    scale = 1.0 / np.sqrt(DH)
    in_maps = []
    for c in range(NCORES):
        hsl = slice(c * HPC * DH, (c + 1) * HPC * DH)
        wq_c = np.asarray(wq)[hsl, :].T * scale  # (1024, 128)
        wk_c = np.asarray(wk)[hsl, :].T
        wv_c = np.asarray(wv)[hsl, :].T
        wqkv = np.concatenate(
            [
                np.stack([wq_c[dc * 128 : (dc + 1) * 128] for dc in range(ND)]),
                np.stack([wk_c[dc * 128 : (dc + 1) * 128] for dc in range(ND)]),
                np.stack([wv_c[dc * 128 : (dc + 1) * 128] for dc in range(ND)]),
            ],
            axis=2,
        )  # (ND, 128, 384)
        m = {
            "xp": xp,
            "wqkv": np.ascontiguousarray(
                wqkv.transpose(1, 0, 2).reshape(128, ND * 384)
            ).astype(BF),
            "bqk": np.ascontiguousarray(
                np.stack([np.asarray(bq)[hsl] * scale, np.asarray(bk)[hsl]], axis=1)
            ).astype(np.float32),
            "bvo": np.asarray(bv)[hsl].reshape(1, 128).astype(BF),
            "wot": np.ascontiguousarray(np.asarray(wo)[:, hsl].T).astype(BF),
        }
        if n_mixed:
            m["mblk"] = mb
        in_maps.append(m)

    res = run_bass_kernel_spmd(nc, in_maps, core_ids=list(range(NCORES)))
    out = np.zeros((128, (T // 128) * D), dtype=np.float64)
    for c in range(NCORES):
        out += res.results[c]["out"].astype(np.float64)
    # unpack row-block-major (128, 32*1024) -> (T, D)
    out = out.reshape(128, T // 128, D).transpose(1, 0, 2).reshape(T, D)
    out = (out + np.asarray(bo)).astype(np.float32)
    return out.reshape(B, S, D)


# revision 18
# speedup vs baseline: 1.1370x; 1.0145x over previous
"""Multi-head attention (B=2, S=2048, D=1024, H=16, causal) on 8 TRN2 cores,
head-parallel: each core computes 2 heads' q/k/v + attention and a partial
output projection; host sums the 8 partials and adds bo.

v2 (bf16): all matmul operands are bf16 (cost-model rate 1 cycle/row at any
moving size; rel-err budget 2e-2 >> bf16 error). HBM traffic is halved and
packed into a handful of large contiguous DMAs via host-side layout:

  xp   (128, 8*8*512)  x^T packed chunk-major: col = t*4096 + dc*512 + c
  wqkv (128, 8*384)    per-dc blocks [wq/sqrt(dh) | wk | wv] columns
  out  (128, 32*1024)  row-block-major: col = blk*1024 + d, blk = token//128

V is projected in flipped orientation (stationary = x sub-chunk, moving = wv
columns) so it lands directly in the [key, dim] layout attention needs - no
PE transposes. Its bias is folded in as a K=1 ones-row matmul. Scores are
computed transposed (S^T[k, q]) per head into halves of one [128,1024] psum
tile so exp / causal-select run once per k-block pair. Normalization: 1/denom
rows (from a ones-column in vaug) broadcast via a tiny ones-matmul, one fused
multiply per chunk; the k-block order puts a full-width block first (psum
start covers every column) and full-width last where possible (clean stop).
"""

import numpy as np
import ml_dtypes

import concourse.bass as bass
import concourse.tile as tile
from concourse import bacc, mybir
from concourse.bass_utils import run_bass_kernel_spmd

B, S, D, H = 2, 2048, 1024, 16
DH = D // H  # 64
NCORES = 8
HPC = H // NCORES  # 2 heads per core
T = B * S  # 4096
QCH = 512
KCH = 128
NQC = S // QCH  # 4
NKC = S // KCH  # 16
NTC = T // QCH  # 8
ND = D // 128  # 8
XC = ND * QCH  # 4096 packed-x columns per token chunk
VW = 65  # vaug block width (64 dims + ones column)

f32 = mybir.dt.float32
bf16 = mybir.dt.bfloat16
AF = mybir.ActivationFunctionType
ALU = mybir.AluOpType
BF = ml_dtypes.bfloat16

PIPE = 4  # score->exp->PV pipeline depth in k-blocks


def _classify_blocks(mask):
    """mask: (S, S) bool [q, k] -> dict (qc, kc) -> (kind, mixed_idx)."""
    blocks = {}
    qg, kg = np.meshgrid(np.arange(S), np.arange(S), indexing="ij")
    causal = qg >= kg
    n_mixed = 0
    for qc in range(NQC):
        for kc in range(NKC):
            reg = mask[qc * QCH : (qc + 1) * QCH, kc * KCH : (kc + 1) * KCH]
            if not reg.any():
                blocks[(qc, kc)] = ("none", -1)
            elif reg.all():
                blocks[(qc, kc)] = ("all", -1)
            elif np.array_equal(
                reg, causal[qc * QCH : (qc + 1) * QCH, kc * KCH : (kc + 1) * KCH]
            ):
                blocks[(qc, kc)] = ("causal", -1)
            else:
                blocks[(qc, kc)] = ("mixed", n_mixed)
                n_mixed += 1
    return blocks, n_mixed


def _order_kcs(blocks, qc):
    """k-block emission order: a full-width block first (its psum write starts
    every column), full-width blocks in the middle, and when possible a
    full-width block last (clean accumulation-group stop)."""
    kcs = [kc for kc in range(NKC) if blocks[(qc, kc)][0] != "none"]
    if not kcs:
        return []

    def f0_of(kc):
        kind, _ = blocks[(qc, kc)]
        return max(0, kc * KCH - qc * QCH) if kind == "causal" else 0

    full = [kc for kc in kcs if f0_of(kc) == 0]
    trimmed = sorted((kc for kc in kcs if f0_of(kc) > 0), key=f0_of, reverse=True)
    assert full, f"q-chunk {qc} has no full-width block"
    if len(full) == 1:
        return [full[0]] + trimmed  # sloppy stop (skip_group_check)
    return full[:-1] + trimmed + [full[-1]]


def _build(mask, reps=1):
    blocks, n_mixed = _classify_blocks(mask)

    nc = bacc.Bacc("TRN2", target_bir_lowering=False, debug=False, num_devices=NCORES)
    x_d = nc.dram_tensor("xp", (128, NTC * XC), bf16, kind="ExternalInput").ap()
    w_d = nc.dram_tensor("wqkv", (128, ND * 384), bf16, kind="ExternalInput").ap()
    bqk_d = nc.dram_tensor("bqk", (128, 2), f32, kind="ExternalInput").ap()
    bvo_d = nc.dram_tensor("bvo", (1, 128), bf16, kind="ExternalInput").ap()
    wo_d = nc.dram_tensor("wot", (128, D), bf16, kind="ExternalInput").ap()
    out_d = nc.dram_tensor("out", (128, (T // 128) * D), bf16, kind="ExternalOutput").ap()
    if n_mixed:
        mb_d = nc.dram_tensor("mblk", (n_mixed * 128, QCH), bf16, kind="ExternalInput").ap()

    def pair_ap(t, f0, width):
        """[128, (2 heads, width)] view of a [128, 1024] tile at column f0."""
        return bass.AP(t.tensor, t.offset + f0, [t.ap[0], [512, 2], [1, width]])

    with tile.TileContext(nc) as tc:
        with (
            tc.tile_pool(name="const", bufs=1) as cpool,
            tc.tile_pool(name="work", bufs=1) as wpool,
            tc.tile_pool(name="psum", bufs=1, space="PSUM") as ppool,
        ):
            # ---- input stream: interleave weights and early x so the first
            # projection matmuls start as soon as (w-dc0/1, x-chunk0) land ----
            w = cpool.tile([128, ND * 384], bf16, name="w")
            xall = cpool.tile([128, NTC * XC], bf16, name="xall")
            nc.sync.dma_start(w[:, 0:768], w_d[:, 0:768])
            nc.sync.dma_start(xall[:, 0:1024], x_d[:, 0:1024])
            nc.sync.dma_start(w[:, 768:1536], w_d[:, 768:1536])
            nc.sync.dma_start(xall[:, 1024:2048], x_d[:, 1024:2048])
            nc.sync.dma_start(w[:, 1536:3072], w_d[:, 1536:3072])
            nc.sync.dma_start(xall[:, 2048:4096], x_d[:, 2048:4096])
            bqk = cpool.tile([128, 2], f32, name="bqk")
            nc.sync.dma_start(bqk[:], bqk_d)
            bvo = cpool.tile([1, 128], bf16, name="bvo")
            nc.sync.dma_start(bvo[:], bvo_d)
            wot = cpool.tile([128, D], bf16, name="wot")
            nc.sync.dma_start(wot[:], wo_d)
            for t in range(1, NTC):
                nc.sync.dma_start(
                    xall[:, t * XC : (t + 1) * XC], x_d[:, t * XC : (t + 1) * XC]
                )

            ones1 = cpool.tile([1, 128], bf16, name="ones1")
            nc.vector.memset(ones1[:], 1.0)
            ones64 = cpool.tile([1, 64], bf16, name="ones64")
            nc.vector.memset(ones64[:], 1.0)

            # ---- per-batch persistent activations ----
            qT = [cpool.tile([128, S], bf16, name=f"qT{b}") for b in range(B)]
            kT = [cpool.tile([128, S], bf16, name=f"kT{b}") for b in range(B)]
            # vaug[b]: h-major [128 keys, 2 * 16 * 65]; col 64 of each
            # 65-block is the ones column producing softmax denominators
            vaug = [cpool.tile([128, HPC * NKC * VW], bf16, name=f"va{b}") for b in range(B)]
            for b in range(B):
                nc.vector.memset(vaug[b][:, 64::VW], 1.0)

            def vslice(b, h, kc):
                return vaug[b][:, h * NKC * VW + kc * VW : h * NKC * VW + kc * VW + VW]

            for _rep in range(reps):
                # ---- phase A units (fine-grained, drained into the k-loops a
                # sub-microsecond piece at a time). q, k, v projections reuse
                # ONE [128,512] psum tile sequentially (WAR on the preceding
                # move orders them); chunk 0's q ladder streams behind the x
                # DMAs ----
                cells = {}  # t -> proj psum tile

                def unit_q_mm(t, half):
                    if half == 0:
                        cells[t] = ppool.tile(
                            [128, 512], f32, tag="proj", bufs=2, name=f"pj{t}"
                        )
                    ps = cells[t]
                    for dc in range(half * 4, half * 4 + 4):
                        nc.tensor.matmul(
                            ps[:],
                            w[:, dc * 384 : dc * 384 + 128],
                            xall[:, t * XC + dc * 512 : t * XC + (dc + 1) * 512],
                            start=(dc == 0),
                            stop=(dc == ND - 1),
                        )

                def unit_k_mm(t, half):
                    ps = cells[t]
                    for dc in range(half * 4, half * 4 + 4):
                        nc.tensor.matmul(
                            ps[:],
                            w[:, dc * 384 + 128 : dc * 384 + 256],
                            xall[:, t * XC + dc * 512 : t * XC + (dc + 1) * 512],
                            start=(dc == 0),
                            stop=(dc == ND - 1),
                        )

                def unit_v_mm(t, half):
                    ps = cells[t]
                    for dc in range(half * 4, half * 4 + 4):
                        for j in range(4):
                            nc.tensor.matmul(
                                ps[:, j * 128 : (j + 1) * 128],
                                xall[:, t * XC + dc * 512 + j * 128 : t * XC + dc * 512 + (j + 1) * 128],
                                w[:, dc * 384 + 256 : dc * 384 + 384],
                                start=(dc == 0),
                                stop=False,
                            )
                    if half == 1:
                        for j in range(4):
                            nc.tensor.matmul(
                                ps[:, j * 128 : (j + 1) * 128], ones1[:], bvo[:],
                                start=False, stop=True,
                            )

                def unit_qmove(t):
                    b, tq = t // NQC, t % NQC
                    nc.vector.tensor_scalar_add(
                        qT[b][:, tq * 512 : (tq + 1) * 512], cells[t][:], bqk[:, 0:1]
                    )

                def unit_kmove(t):
                    b, tq = t // NQC, t % NQC
                    nc.vector.tensor_scalar_add(
                        kT[b][:, tq * 512 : (tq + 1) * 512], cells[t][:], bqk[:, 1:2]
                    )

                def unit_vcopy(t):
                    b, tq = t // NQC, t % NQC
                    vp = cells.pop(t)
                    va = vaug[b]
                    dst = bass.AP(
                        va.tensor,
                        va.offset + tq * 4 * VW,
                        [va.ap[0], [NKC * VW, 2], [VW, 4], [1, 64]],
                    )
                    src = bass.AP(
                        vp.tensor, vp.offset, [vp.ap[0], [64, 2], [128, 4], [1, 64]]
                    )
                    nc.vector.tensor_copy(dst, src)

                fill = []  # (chunk, unit_idx, closure)
                N_UNITS = 9
                MOVES_DONE = 6  # units < 6: q/k matmuls + their moves

                def queue_A(t):
                    if t >= NTC:
                        return
                    units = [
                        lambda t=t: unit_q_mm(t, 0),
                        lambda t=t: unit_q_mm(t, 1),
                        lambda t=t: unit_qmove(t),
                        lambda t=t: unit_k_mm(t, 0),
                        lambda t=t: unit_k_mm(t, 1),
                        lambda t=t: unit_kmove(t),
                        lambda t=t: unit_v_mm(t, 0),
                        lambda t=t: unit_v_mm(t, 1),
                        lambda t=t: unit_vcopy(t),
                    ]
                    for u, fn in enumerate(units):
                        fill.append((t, u, fn))

                def need_A(t, n_units):
                    while fill and (
                        fill[0][0] < t or (fill[0][0] == t and fill[0][1] < n_units)
                    ):
                        fill.pop(0)[2]()

                def pop_fill():
                    if fill:
                        fill.pop(0)[2]()
                        return True
                    return False

                queued = [0]

                def ensure_queued(t):
                    while queued[0] <= min(t, NTC - 1):
                        queue_A(queued[0])
                        queued[0] += 1

                # ---- phases B/C per (batch, q-chunk) ----
                pending = []

                def pop_pending():
                    if pending:
                        pending.pop(0)()
                        return True
                    return False

                def flush_pending():
                    while pending:
                        pending.pop(0)()

                for b in range(B):
                    for qc in range(NQC):
                        kcs = _order_kcs(blocks, qc)
                        kmax = max(kcs) // (QCH // KCH) if kcs else 0
                        req = b * NQC + max(qc, kmax)
                        ensure_queued(req)
                        need_A(req, MOVES_DONE)  # qT/kT of this chunk before scores
                        # first k-block index (emission order) whose PV reads
                        # vaug written by this chunk's own phase A
                        first_own = min(
                            (i for i, kc in enumerate(kcs) if kc // (QCH // KCH) >= qc),
                            default=len(kcs),
                        )
                        acc = ppool.tile(
                            [128, 1024], f32, tag="acc", bufs=1, name=f"acc{b}_{qc}"
                        )
                        pts = {}
                        sloppy_stop = blocks[(qc, kcs[-1])][0] == "causal" and (
                            kcs[-1] * KCH > qc * QCH
                        )

                        def emit_scores(i, b=b, qc=qc, kcs=kcs, pts=pts):
                            kc = kcs[i]
                            kind, midx = blocks[(qc, kc)]
                            f0 = 0
                            if kind == "causal":
                                f0 = max(0, kc * KCH - qc * QCH)
                            st = ppool.tile(
                                [128, 1024], f32, tag="st", bufs=2, name=f"st{b}_{qc}_{i}"
                            )
                            for h in range(HPC):
                                nc.tensor.matmul(
                                    st[:, h * 512 + f0 : (h + 1) * 512],
                                    kT[b][h * 64 : (h + 1) * 64, kc * KCH : (kc + 1) * KCH],
                                    qT[b][h * 64 : (h + 1) * 64, qc * QCH + f0 : (qc + 1) * QCH],
                                    start=True,
                                    stop=True,
                                    tile_position=(h * 64, 0),
                                )
                            pt = wpool.tile(
                                [128, 1024], bf16, tag="pt", bufs=8, name=f"pt{b}_{qc}_{i}"
                            )
                            if f0:
                                nc.scalar.activation(
                                    pair_ap(pt, f0, 512 - f0), pair_ap(st, f0, 512 - f0), AF.Exp
                                )
                            else:
                                nc.scalar.activation(pt[:], st[:], AF.Exp)
                            if kind == "causal":
                                nc.gpsimd.affine_select(
                                    out=pair_ap(pt, f0, 512 - f0),
                                    in_=pair_ap(pt, f0, 512 - f0),
                                    compare_op=ALU.is_ge,
                                    fill=0.0,
                                    base=qc * QCH - kc * KCH + f0,
                                    pattern=[[0, 2], [1, 512 - f0]],
                                    channel_multiplier=-1,
                                )
                            elif kind == "mixed":
                                mt = wpool.tile(
                                    [128, QCH], bf16, tag="mt", bufs=4, name=f"mt{b}_{qc}_{i}"
                                )
                                nc.sync.dma_start(mt[:], mb_d[midx * 128 : (midx + 1) * 128, :])
                                for h in range(HPC):
                                    nc.vector.tensor_mul(
                                        pt[:, h * 512 : (h + 1) * 512],
                                        pt[:, h * 512 : (h + 1) * 512],
                                        mt[:],
                                    )
                            pts[(i,)] = (pt, f0)

                        def emit_pv(i, b=b, qc=qc, kcs=kcs, pts=pts, acc=acc,
                                    sloppy=sloppy_stop, first_own=first_own, req=req):
                            if i >= first_own:
                                need_A(req, N_UNITS)  # vaug of this chunk before own-PV
                            kc = kcs[i]
                            pt, f0 = pts.pop((i,))
                            last = i == len(kcs) - 1
                            for h in range(HPC):
                                nc.tensor.matmul(
                                    acc[0:65, h * 512 + f0 : (h + 1) * 512],
                                    vslice(b, h, kc),
                                    pt[:, h * 512 + f0 : (h + 1) * 512],
                                    start=(i == 0),
                                    stop=last,
                                    skip_group_check=(f0 > 0 or (last and sloppy)),
                                )

                        ensure_queued(req + 1)
                        for i in range(len(kcs)):
                            emit_scores(i)
                            if i >= PIPE:
                                emit_pv(i - PIPE)
                            # drain next chunk's projections first (their
                            # qT/kT feed the next loop), then prior chunk's
                            # deferred normalize/out-projection (not before
                            # i==4: its reciprocal needs a DVE-queue head
                            # start)
                            if i >= 1:
                                if not (pop_fill() and pop_fill()) and i >= 4:
                                    pop_pending()
                        for i in range(max(0, len(kcs) - PIPE), len(kcs)):
                            emit_pv(i)
                            pop_pending()

                        # ---- phase C: reciprocal + unnormalized attnT now;
                        # broadcast/normalize/out-proj deferred into the next
                        # chunk's k-loop as five sub-microsecond pieces ----
                        rec = wpool.tile([1, 1024], bf16, tag="rec", bufs=2, name=f"rc{b}{qc}")
                        with nc.allow_low_precision(reason="softmax 1/denom in bf16"):
                            nc.vector.reciprocal(
                                rec[:],
                                bass.AP(
                                    acc.tensor,
                                    acc.offset + 64 * acc.ap[0][0],
                                    [[acc.ap[0][0], 1], [1, 1024]],
                                ),
                            )
                        attnT = wpool.tile(
                            [128, QCH], bf16, tag="attnT", bufs=2, name=f"at{b}_{qc}"
                        )
                        for h in range(HPC):
                            nc.vector.tensor_copy(
                                attnT[h * 64 : (h + 1) * 64, :],
                                acc[0:64, h * 512 : (h + 1) * 512],
                            )
                        flush_pending()  # any leftover phase C of the prior chunk

                        _last = b == B - 1 and qc == NQC - 1
                        osb = wpool.tile(
                            [128, 4096], bf16, tag="osb", bufs=2, name=f"ob{b}_{qc}"
                        )

                        def sub_norm(b=b, qc=qc, attnT=attnT, rec=rec):
                            bcb = wpool.tile(
                                [128, 1024], bf16, tag="bcb", bufs=2, name=f"bc{b}_{qc}"
                            )
                            nc.gpsimd.partition_broadcast(bcb[:], rec[:])
                            for h in range(HPC):
                                nc.vector.tensor_mul(
                                    attnT[h * 64 : (h + 1) * 64, :],
                                    attnT[h * 64 : (h + 1) * 64, :],
                                    bcb[h * 64 : (h + 1) * 64, h * 512 : (h + 1) * 512],
                                )

                        def sub_oproj(tk, b=b, qc=qc, attnT=attnT, osb=osb, _last=_last):
                            blk0 = (b * S + qc * QCH) // 128
                            op = ppool.tile(
                                [128, 1024], f32, tag="st", bufs=2, name=f"op{b}_{qc}_{tk}"
                            )
                            for oc in range(2):
                                nc.tensor.matmul(
                                    op[:, oc * 512 : (oc + 1) * 512],
                                    attnT[:, tk * 128 : (tk + 1) * 128],
                                    wot[:, oc * 512 : (oc + 1) * 512],
                                    start=True,
                                    stop=True,
                                )
                            dst = osb[:, tk * 1024 : (tk + 1) * 1024]
                            if tk == 2:
                                nc.scalar.copy(dst, op[:])
                            elif tk % 2:
                                nc.gpsimd.tensor_copy(dst, op[:])
                            else:
                                nc.vector.tensor_copy(dst, op[:])
                            if _last:
                                nc.sync.dma_start(
                                    out_d[:, (blk0 + tk) * 1024 : (blk0 + tk + 1) * 1024],
                                    osb[:, tk * 1024 : (tk + 1) * 1024],
                                )
                            elif tk == 3:
                                nc.sync.dma_start(
                                    out_d[:, blk0 * 1024 : (blk0 + 4) * 1024], osb[:]
                                )

                        pending.append(sub_norm)
                        for tk in range(4):
                            pending.append(lambda tk=tk: sub_oproj(tk))
                flush_pending()

    nc.compile()
    return nc, blocks, n_mixed


_CACHE = {}


def _get_program(mask):
    key = mask.tobytes()
    if key not in _CACHE:
        _CACHE[key] = _build(mask)
    return _CACHE[key]


def kernel(x, mask, wq, bq, wk, bk, wv, bv, wo, bo):
    x = np.asarray(x, dtype=np.float32)
    mask2 = np.asarray(mask).reshape(S, S)
    nc, blocks, n_mixed = _get_program(mask2)

    # pack x^T chunk-major: xp[p, t*4096 + dc*512 + c] = x[token t*512+c, dc*128+p]
    xp = np.ascontiguousarray(
        x.reshape(NTC, QCH, ND, 128).transpose(3, 0, 2, 1).reshape(128, NTC * XC)
    ).astype(BF)

    if n_mixed:
        mb = np.zeros((n_mixed * 128, QCH), dtype=BF)
        for (qc, kc), (kind, midx) in blocks.items():
            if kind == "mixed":
                reg = mask2[qc * QCH : (qc + 1) * QCH, kc * KCH : (kc + 1) * KCH]
                mb[midx * 128 : (midx + 1) * 128, :] = reg.T.astype(BF)

    scale = 1.0 / np.sqrt(DH)
    in_maps = []
    for c in range(NCORES):
        hsl = slice(c * HPC * DH, (c + 1) * HPC * DH)
        wq_c = np.asarray(wq)[hsl, :].T * scale  # (1024, 128)
        wk_c = np.asarray(wk)[hsl, :].T
        wv_c = np.asarray(wv)[hsl, :].T
        wqkv = np.concatenate(
            [
                np.stack([wq_c[dc * 128 : (dc + 1) * 128] for dc in range(ND)]),
                np.stack([wk_c[dc * 128 : (dc + 1) * 128] for dc in range(ND)]),
                np.stack([wv_c[dc * 128 : (dc + 1) * 128] for dc in range(ND)]),
            ],
            axis=2,
        )  # (ND, 128, 384)
        m = {
            "xp": xp,
            "wqkv": np.ascontiguousarray(
                wqkv.transpose(1, 0, 2).reshape(128, ND * 384)
            ).astype(BF),
            "bqk": np.ascontiguousarray(
                np.stack([np.asarray(bq)[hsl] * scale, np.asarray(bk)[hsl]], axis=1)
            ).astype(np.float32),
            "bvo": np.asarray(bv)[hsl].reshape(1, 128).astype(BF),
            "wot": np.ascontiguousarray(np.asarray(wo)[:, hsl].T).astype(BF),
        }
        if n_mixed:
            m["mblk"] = mb
        in_maps.append(m)

    res = run_bass_kernel_spmd(nc, in_maps, core_ids=list(range(NCORES)))
    out = np.zeros((128, (T // 128) * D), dtype=np.float64)
    for c in range(NCORES):
        out += res.results[c]["out"].astype(np.float64)
    # unpack row-block-major (128, 32*1024) -> (T, D)
    out = out.reshape(128, T // 128, D).transpose(1, 0, 2).reshape(T, D)
    out = (out + np.asarray(bo)).astype(np.float32)
    return out.reshape(B, S, D)
